# revision 22
# baseline (speedup 1.0000x reference)
"""Quantized Linear (8-bit act / 4-bit weight fake-quant) on 8 Trainium2 cores.

Math (per reference):
  xq = rne(x / s_x) * s_x          s_x = max(absmax(x)/127, 1e-8)
  wq = rne(w / s_w) * s_w          s_w = max(absmax(w)/7,   1e-8)
  bq = rne(b / s_b) * s_b          s_b = max(absmax(b)/127, 1e-8)
  out_pre = bq + xq @ wq.T
  out = rne(out_pre / s_o) * s_o   s_o = max(absmax(out_pre)/127, 1e-8)

Device strategy (4x2 grid: tokens x out_features, 8 cores):
  - Host feeds each core k-major slabs (xT slices; w as [m, k, kt, j] blocks
    so each 2-MiB weight chunk moves with 16-KB descriptors). No PE
    transposes: quantized tiles land directly in [k_partition, free] layout
    (lhsT = w tile, rhs = xT strip, out = outT).
  - Quantization uses the hardware fp32->int16 convert (round-to-nearest-
    even, matches jnp.round): pass1 = x*inv_s -> int16, pass2 = int16 ->
    bf16 copy (exact for |q| <= 127). Integer bf16 matmul accumulates
    exactly in fp32 PSUM; scales fold into the PSUM eviction.
  - Per-core exclusive absmax slices (first 512 columns of each slab, made
    uniform across cores by host-side np.roll) + one tiny AllReduce-max for
    (x, w); a second one for out_pre before the final requantization.
  - Each core computes outT[j_shard, t_shard] = [2048, 1024]; host
    un-rolls and reassembles.
"""

import sys

sys.path.insert(0, "/opt/trn_rl_repo")

import numpy as np

import concourse.bass as bass
import concourse.mybir as mybir
import concourse.tile as tile
from concourse import bacc, bass_isa

F32 = mybir.dt.float32
BF16 = mybir.dt.bfloat16
I16 = mybir.dt.int16
AF = mybir.ActivationFunctionType
ALU = mybir.AluOpType
AX = mybir.AxisListType

EPS = 1e-8
INV_QA = float(np.float32(1.0) / np.float32(127.0))
INV_QW = float(np.float32(1.0) / np.float32(7.0))

P = 128
N_TP = 4  # token-parallel degree
N_JP = 2  # feature-parallel degree


def build(n_cores=8, T=4096, K=4096, J=4096, TB=512):
    assert n_cores == N_TP * N_JP
    TS, JS = T // N_TP, J // N_JP  # 1024 tokens, 2048 features per core
    n_kt = K // P  # 32 contraction tiles
    n_m = JS // P  # 16 output-feature tiles
    n_tb = TS // TB  # 2 psum column groups

    nc = bacc.Bacc(
        "TRN2", target_bir_lowering=False, debug=False, num_devices=n_cores
    )

    xq_d = nc.dram_tensor("xq", [K, TS], F32, kind="ExternalInput")
    wh_d = nc.dram_tensor("wh", [n_m, P, n_kt, P], F32, kind="ExternalInput")
    b_d = nc.dram_tensor("b_full", [J], F32, kind="ExternalInput")
    bs_d = nc.dram_tensor("b_shard", [JS], F32, kind="ExternalInput")
    o_d = nc.dram_tensor("outT", [JS, TS], F32, kind="ExternalOutput")
    cc1_in = nc.dram_tensor("cc1_in", [1, 3], F32)
    cc1_out = nc.dram_tensor("cc1_out", [1, 3], F32)
    cc2_in = nc.dram_tensor("cc2_in", [1, 1], F32)
    cc2_out = nc.dram_tensor("cc2_out", [1, 1], F32)
    groups = [list(range(n_cores))]

    xr = xq_d.rearrange("(kt k) t -> k kt t", k=P)

    with tile.TileContext(nc) as tc:
        with (
            tc.tile_pool(name="big", bufs=1) as big,
            tc.tile_pool(name="scal", bufs=1) as scal,
            tc.tile_pool(name="xslp", bufs=2) as xslp,
            tc.tile_pool(name="xsp", bufs=2) as xsp,
            tc.tile_pool(name="xmp", bufs=2) as xmp,
            tc.tile_pool(name="wrp", bufs=2) as wrp,
            tc.tile_pool(name="wmp", bufs=2) as wmp,
            tc.tile_pool(name="qwp", bufs=4) as qwp,
            tc.tile_pool(name="mmps", bufs=8, space="PSUM") as mmps,
        ):
            qxT = big.tile([P, n_kt, TS], BF16)  # 64 KB/partition
            opre = big.tile([P, n_m, TS], F32)  # 64 KB/partition

            # ---------------- Phase 0: absmax of exclusive slices ----------
            # Host rolls each slab so this core's exclusive slice is always
            # columns [0, 512) of xq and chunks [0, 4) of wh.
            HC = n_kt // 2  # w half-chunk: 16 kt rows
            whr = wh_d.rearrange("m k (h kt) j -> m k h kt j", h=2)
            n_xs = 16  # x-slice strips of [128, 2kt, 512]
            am = scal.tile([P, n_xs + 8 + 1], F32)
            xsl = xr[:, :, 0:512].rearrange("k (g t) c -> k g t c", g=n_xs)
            for g in range(n_xs):
                t = xslp.tile([P, 2, 512], F32, tag="xs", name=f"xs{g}")
                eng = nc.sync if g % 2 == 0 else nc.gpsimd
                eng.dma_start(t[:], xsl[:, g])
                if g < 8:
                    w = wrp.tile([P, HC, P], F32, tag="wr", name=f"ws{g}")
                    nc.scalar.dma_start(w[:], whr[g // 2, :, g % 2])
                    # interleave x/w reduces: ready w data fills x-DMA waits
                    nc.vector.tensor_reduce(
                        am[:, n_xs + g : n_xs + g + 1],
                        w[:].rearrange("k a b -> k (a b)"),
                        axis=AX.X, op=ALU.max, apply_absolute_value=True,
                    )
                nc.vector.tensor_reduce(
                    am[:, g : g + 1], t[:].rearrange("k a b -> k (a b)"),
                    axis=AX.X, op=ALU.max, apply_absolute_value=True,
                )
            bfull = scal.tile([P, J // P], F32)
            nc.gpsimd.dma_start(bfull[:], b_d.rearrange("(p a) -> p a", p=P))
            nc.vector.tensor_reduce(
                am[:, n_xs + 8 :], bfull[:], axis=AX.X, op=ALU.max,
                apply_absolute_value=True,
            )

            m3 = scal.tile([P, 3], F32)
            nc.vector.tensor_reduce(
                m3[:, 0:1], am[:, 0:n_xs], axis=AX.X, op=ALU.max
            )
            nc.vector.tensor_reduce(
                m3[:, 1:2], am[:, n_xs : n_xs + 8], axis=AX.X, op=ALU.max
            )
            nc.vector.tensor_copy(out=m3[:, 2:3], in_=am[:, n_xs + 8 :])
            g3 = scal.tile([P, 3], F32)
            nc.gpsimd.partition_all_reduce(
                g3[:], m3[:], channels=P, reduce_op=bass_isa.ReduceOp.max
            )
            nc.sync.dma_start(cc1_in[:], g3[:1, :])
            nc.gpsimd.collective_compute(
                "AllReduce", ALU.max, replica_groups=groups,
                ins=[cc1_in[:]], outs=[cc1_out[:]],
            )
            gx = scal.tile([P, 3], F32)
            nc.sync.dma_start(gx[:1, :], cc1_out[:])
            bc3 = scal.tile([P, 3], F32)
            nc.gpsimd.partition_broadcast(bc3[:], gx[:1, :], channels=P)

            # fused scale chain, single engine (DVE), minimal cross-engine hops
            cmul3 = scal.tile([P, 3], F32)
            nc.vector.memset(cmul3[:, 0:1], INV_QA)
            nc.vector.memset(cmul3[:, 1:2], INV_QW)
            nc.vector.memset(cmul3[:, 2:3], INV_QA)
            s3 = scal.tile([P, 3], F32)
            nc.vector.tensor_tensor(out=s3[:], in0=bc3[:], in1=cmul3[:], op=ALU.mult)
            nc.vector.tensor_scalar(s3[:], s3[:], EPS, None, op0=ALU.max)
            inv3 = scal.tile([P, 3], F32)
            nc.vector.reciprocal(inv3[:], s3[:])
            s_x, s_w, s_b = s3[:, 0:1], s3[:, 1:2], s3[:, 2:3]
            inv_sx, inv_sw, inv_sb = inv3[:, 0:1], inv3[:, 1:2], inv3[:, 2:3]
            s_xw = scal.tile([P, 1], F32)
            nc.vector.tensor_tensor(out=s_xw[:], in0=s_x, in1=s_w, op=ALU.mult)

            bsh = scal.tile([P, n_m], F32)
            nc.gpsimd.dma_start(bsh[:], bs_d.rearrange("(a p) -> p a", p=P))
            bqi = scal.tile([P, n_m], I16)
            nc.scalar.activation(bqi[:], bsh[:], AF.Identity, scale=inv_sb)
            bq = scal.tile([P, n_m], F32)
            nc.scalar.activation(bq[:], bqi[:], AF.Identity, scale=s_b)

            # ---------------- Phase 1: quantize x -> qxT (bf16) ------------
            # kt-major strips [128, 1024]; pass1 alternates ACT/DVE, pass2 DVE.
            for kt in range(n_kt):
                t = xsp.tile([P, TS], F32, tag="xf", name=f"xf{kt}")
                nc.sync.dma_start(t[:], xr[:, kt, :])
                mid = xmp.tile([P, TS], I16, tag="xm", name=f"xm{kt}")
                if kt % 2 == 0:
                    nc.scalar.activation(mid[:], t[:], AF.Identity, scale=inv_sx)
                else:
                    nc.vector.tensor_scalar(mid[:], t[:], inv_sx, None, op0=ALU.mult)
                nc.vector.tensor_copy(out=qxT[:, kt, :], in_=mid[:])

            # ---------------- Phase 2: stream w, quantize, matmul ----------
            def w_prep(m):
                qw = qwp.tile([P, n_kt, P], BF16, tag="qw", name=f"qw{m}")
                for h in range(2):
                    t = wrp.tile([P, HC, P], F32, tag="wr", name=f"wc{m}_{h}")
                    nc.sync.dma_start(t[:], whr[m, :, h])
                    mid = wmp.tile([P, HC, P], I16, tag="wm", name=f"wm{m}_{h}")
                    nc.scalar.activation(
                        mid[:].rearrange("k a b -> k (a b)"),
                        t[:].rearrange("k a b -> k (a b)"),
                        AF.Identity, scale=inv_sw,
                    )
                    nc.vector.tensor_copy(
                        out=qw[:, h * HC : (h + 1) * HC, :], in_=mid[:]
                    )
                return qw

            def evict(m, ps):
                for tb in range(n_tb):
                    nc.scalar.activation(
                        opre[:, m, tb * TB : (tb + 1) * TB], ps[tb][:],
                        AF.Identity, bias=bq[:, m : m + 1], scale=s_xw[:],
                    )

            # First 4 m-tiles run kt-outer across all 8 PSUM banks: PE demand
            # per x-strip stays below the strip DMA rate, so the PE does not
            # stall while qxT streams in.
            NB = 0
            qws = [w_prep(m) for m in range(NB)]
            psb = [
                [mmps.tile([P, TB], F32, tag="mm", name=f"psb{m}_{tb}") for tb in range(n_tb)]
                for m in range(NB)
            ]
            for kt in range(n_kt):
                for m in range(NB):
                    for tb in range(n_tb):
                        nc.tensor.matmul(
                            psb[m][tb][:],
                            lhsT=qws[m][:, kt, :],
                            rhs=qxT[:, kt, tb * TB : (tb + 1) * TB],
                            start=(kt == 0),
                            stop=(kt == n_kt - 1),
                        )
            for m in range(NB):
                evict(m, psb[m])

            for m in range(NB, n_m):
                qw = w_prep(m)
                ps = [
                    mmps.tile([P, TB], F32, tag="mm", name=f"ps{m}_{tb}")
                    for tb in range(n_tb)
                ]
                for kt in range(n_kt):
                    for tb in range(n_tb):
                        nc.tensor.matmul(
                            ps[tb][:],
                            lhsT=qw[:, kt, :],
                            rhs=qxT[:, kt, tb * TB : (tb + 1) * TB],
                            start=(kt == 0),
                            stop=(kt == n_kt - 1),
                        )
                evict(m, ps)

            omax = scal.tile([P, n_m], F32)
            for m in range(n_m):
                nc.vector.tensor_reduce(
                    omax[:, m : m + 1], opre[:, m, :], axis=AX.X, op=ALU.max,
                    apply_absolute_value=True,
                )

            # ---------------- Phase 3: out absmax -> requantize ------------
            om1 = scal.tile([P, 1], F32)
            nc.vector.tensor_reduce(om1[:], omax[:], axis=AX.X, op=ALU.max)
            omr = scal.tile([P, 1], F32)
            nc.gpsimd.partition_all_reduce(
                omr[:], om1[:], channels=P, reduce_op=bass_isa.ReduceOp.max
            )
            nc.sync.dma_start(cc2_in[:], omr[:1, :])
            nc.gpsimd.collective_compute(
                "AllReduce", ALU.max, replica_groups=groups,
                ins=[cc2_in[:]], outs=[cc2_out[:]],
            )
            go = scal.tile([P, 1], F32)
            nc.sync.dma_start(go[:1, :], cc2_out[:])
            bco = scal.tile([P, 1], F32)
            nc.gpsimd.partition_broadcast(bco[:], go[:1, :], channels=P)
            s_o = scal.tile([P, 1], F32)
            nc.vector.tensor_scalar(s_o[:], bco[:], INV_QA, EPS, op0=ALU.mult, op1=ALU.max)
            inv_so = scal.tile([P, 1], F32)
            nc.vector.reciprocal(inv_so[:], s_o[:])

            for m in range(n_m):
                oi = xmp.tile([P, TS], I16, tag="xm", name=f"oi{m}")
                nc.vector.tensor_scalar(oi[:], opre[:, m, :], inv_so[:], None, op0=ALU.mult)
                # requantized values overwrite opre in place, then stream out
                nc.scalar.activation(opre[:, m, :], oi[:], AF.Identity, scale=s_o[:])
                nc.sync.dma_start(o_d[m * P : (m + 1) * P, :], opre[:, m, :])

    nc.compile()
    return nc


def _run(nc, inputs, n_cores, T, K, J, trace=False):
    from concourse.bass_utils import run_bass_kernel_spmd

    TS, JS = T // N_TP, J // N_JP
    n_m = JS // P
    n_kt = K // P
    x = np.ascontiguousarray(inputs["x"], dtype=np.float32)
    w = np.ascontiguousarray(inputs["weight"], dtype=np.float32)
    b = np.ascontiguousarray(inputs["b"], dtype=np.float32)
    xT = np.ascontiguousarray(x.T)  # [K, T]
    in_maps = []
    for c in range(n_cores):
        ti, jb = c // N_JP, c % N_JP
        # roll so the core's exclusive absmax slice is always columns [0,512)
        xs = np.roll(xT[:, ti * TS : (ti + 1) * TS], -512 * jb, axis=1)
        wsh = np.roll(w[jb * JS : (jb + 1) * JS, :], -512 * ti, axis=0)
        # [m, k, kt, j]: chunk m contiguous, 16-KB runs per partition k
        wh = np.ascontiguousarray(
            wsh.reshape(n_m, P, n_kt, P).transpose(0, 3, 2, 1)
        )
        in_maps.append(
            {
                "xq": np.ascontiguousarray(xs),
                "wh": wh,
                "b_full": b,
                "b_shard": np.ascontiguousarray(
                    np.roll(b[jb * JS : (jb + 1) * JS], -512 * ti)
                ),
            }
        )
    res = run_bass_kernel_spmd(nc, in_maps, core_ids=list(range(n_cores)), trace=trace)
    out = np.empty((T, J), dtype=np.float32)
    for c in range(n_cores):
        ti, jb = c // N_JP, c % N_JP
        oT = res.results[c]["outT"]  # [JS, TS], rolled in both dims
        oT = np.roll(oT, 512 * ti, axis=0)
        oT = np.roll(oT, 512 * jb, axis=1)
        out[ti * TS : (ti + 1) * TS, jb * JS : (jb + 1) * JS] = oT.T
    return out, res


_NC_CACHE = {}


def kernel(**inputs) -> np.ndarray:
    n_cores, T, K, J = 8, 4096, 4096, 4096
    key = (n_cores, T, K, J)
    if key not in _NC_CACHE:
        _NC_CACHE[key] = build(n_cores, T, K, J)
    out, _ = _run(_NC_CACHE[key], inputs, n_cores, T, K, J)
    return out


# revision 23
# speedup vs baseline: 1.0031x; 1.0031x over previous
"""Quantized Linear (8-bit act / 4-bit weight fake-quant) on 8 Trainium2 cores.

Math (per reference):
  xq = rne(x / s_x) * s_x          s_x = max(absmax(x)/127, 1e-8)
  wq = rne(w / s_w) * s_w          s_w = max(absmax(w)/7,   1e-8)
  bq = rne(b / s_b) * s_b          s_b = max(absmax(b)/127, 1e-8)
  out_pre = bq + xq @ wq.T
  out = rne(out_pre / s_o) * s_o   s_o = max(absmax(out_pre)/127, 1e-8)

Device strategy (4x2 grid: tokens x out_features, 8 cores):
  - Host feeds each core k-major slabs (xT slices; w as [m, k, kt, j] blocks
    so each 2-MiB weight chunk moves with 16-KB descriptors). No PE
    transposes: quantized tiles land directly in [k_partition, free] layout
    (lhsT = w tile, rhs = xT strip, out = outT).
  - Quantization uses the hardware fp32->int16 convert (round-to-nearest-
    even, matches jnp.round): pass1 = x*inv_s -> int16, pass2 = int16 ->
    bf16 copy (exact for |q| <= 127). Integer bf16 matmul accumulates
    exactly in fp32 PSUM; scales fold into the PSUM eviction.
  - Per-core exclusive absmax slices (first 512 columns of each slab, made
    uniform across cores by host-side np.roll) + one tiny AllReduce-max for
    (x, w); a second one for out_pre before the final requantization.
  - Each core computes outT[j_shard, t_shard] = [2048, 1024]; host
    un-rolls and reassembles.
"""

import sys

sys.path.insert(0, "/opt/trn_rl_repo")

import numpy as np

import concourse.bass as bass
import concourse.mybir as mybir
import concourse.tile as tile
from concourse import bacc, bass_isa

F32 = mybir.dt.float32
BF16 = mybir.dt.bfloat16
I16 = mybir.dt.int16
AF = mybir.ActivationFunctionType
ALU = mybir.AluOpType
AX = mybir.AxisListType

EPS = 1e-8
INV_QA = float(np.float32(1.0) / np.float32(127.0))
INV_QW = float(np.float32(1.0) / np.float32(7.0))

P = 128
N_TP = 4  # token-parallel degree
N_JP = 2  # feature-parallel degree


def build(n_cores=8, T=4096, K=4096, J=4096, TB=512):
    assert n_cores == N_TP * N_JP
    TS, JS = T // N_TP, J // N_JP  # 1024 tokens, 2048 features per core
    n_kt = K // P  # 32 contraction tiles
    n_m = JS // P  # 16 output-feature tiles
    n_tb = TS // TB  # 2 psum column groups

    nc = bacc.Bacc(
        "TRN2", target_bir_lowering=False, debug=False, num_devices=n_cores
    )

    xq_d = nc.dram_tensor("xq", [K, TS], F32, kind="ExternalInput")
    wh_d = nc.dram_tensor("wh", [n_m, P, n_kt, P], F32, kind="ExternalInput")
    b_d = nc.dram_tensor("b_full", [J], F32, kind="ExternalInput")
    bs_d = nc.dram_tensor("b_shard", [JS], F32, kind="ExternalInput")
    o_d = nc.dram_tensor("outT", [JS, TS], F32, kind="ExternalOutput")
    cc1_in = nc.dram_tensor("cc1_in", [1, 3], F32)
    cc1_out = nc.dram_tensor("cc1_out", [1, 3], F32)
    cc2_in = nc.dram_tensor("cc2_in", [1, 1], F32)
    cc2_out = nc.dram_tensor("cc2_out", [1, 1], F32)
    groups = [list(range(n_cores))]

    xr = xq_d.rearrange("(kt k) t -> k kt t", k=P)

    with tile.TileContext(nc) as tc:
        with (
            tc.tile_pool(name="big", bufs=1) as big,
            tc.tile_pool(name="scal", bufs=1) as scal,
            tc.tile_pool(name="xslp", bufs=2) as xslp,
            tc.tile_pool(name="xsp", bufs=2) as xsp,
            tc.tile_pool(name="xmp", bufs=2) as xmp,
            tc.tile_pool(name="wrp", bufs=2) as wrp,
            tc.tile_pool(name="wmp", bufs=2) as wmp,
            tc.tile_pool(name="qwp", bufs=4) as qwp,
            tc.tile_pool(name="mmps", bufs=8, space="PSUM") as mmps,
        ):
            qxT = big.tile([P, n_kt, TS], BF16)  # 64 KB/partition
            opre = big.tile([P, n_m, TS], F32)  # 64 KB/partition

            # ---------------- Phase 0: absmax of exclusive slices ----------
            # Host rolls each slab so this core's exclusive slice is always
            # columns [0, 512) of xq and chunks [0, 4) of wh.
            HC = n_kt // 2  # w half-chunk: 16 kt rows
            whr = wh_d.rearrange("m k (h kt) j -> m k h kt j", h=2)
            n_xs = 16  # x-slice strips of [128, 2kt, 512]
            am = scal.tile([P, n_xs + 8 + 1], F32)
            xsl = xr[:, :, 0:512].rearrange("k (g t) c -> k g t c", g=n_xs)
            for g in range(n_xs):
                t = xslp.tile([P, 2, 512], F32, tag="xs", name=f"xs{g}")
                eng = nc.sync if g % 2 == 0 else nc.gpsimd
                eng.dma_start(t[:], xsl[:, g])
                if g < 8:
                    w = wrp.tile([P, HC, P], F32, tag="wr", name=f"ws{g}")
                    nc.scalar.dma_start(w[:], whr[g // 2, :, g % 2])
                    # interleave x/w reduces: ready w data fills x-DMA waits
                    nc.vector.tensor_reduce(
                        am[:, n_xs + g : n_xs + g + 1],
                        w[:].rearrange("k a b -> k (a b)"),
                        axis=AX.X, op=ALU.max, apply_absolute_value=True,
                    )
                nc.vector.tensor_reduce(
                    am[:, g : g + 1], t[:].rearrange("k a b -> k (a b)"),
                    axis=AX.X, op=ALU.max, apply_absolute_value=True,
                )
            bfull = scal.tile([P, J // P], F32)
            nc.gpsimd.dma_start(bfull[:], b_d.rearrange("(p a) -> p a", p=P))
            nc.vector.tensor_reduce(
                am[:, n_xs + 8 :], bfull[:], axis=AX.X, op=ALU.max,
                apply_absolute_value=True,
            )

            m3 = scal.tile([P, 3], F32)
            nc.vector.tensor_reduce(
                m3[:, 0:1], am[:, 0:n_xs], axis=AX.X, op=ALU.max
            )
            nc.vector.tensor_reduce(
                m3[:, 1:2], am[:, n_xs : n_xs + 8], axis=AX.X, op=ALU.max
            )
            nc.vector.tensor_copy(out=m3[:, 2:3], in_=am[:, n_xs + 8 :])
            g3 = scal.tile([P, 3], F32)
            nc.gpsimd.partition_all_reduce(
                g3[:], m3[:], channels=P, reduce_op=bass_isa.ReduceOp.max
            )
            nc.sync.dma_start(cc1_in[:], g3[:1, :])
            nc.gpsimd.collective_compute(
                "AllReduce", ALU.max, replica_groups=groups,
                ins=[cc1_in[:]], outs=[cc1_out[:]],
            )
            gx = scal.tile([P, 3], F32)
            nc.sync.dma_start(gx[:1, :], cc1_out[:])
            bc3 = scal.tile([P, 3], F32)
            nc.gpsimd.partition_broadcast(bc3[:], gx[:1, :], channels=P)

            # fused scale chain, single engine (DVE), minimal cross-engine hops
            cmul3 = scal.tile([P, 3], F32)
            nc.vector.memset(cmul3[:, 0:1], INV_QA)
            nc.vector.memset(cmul3[:, 1:2], INV_QW)
            nc.vector.memset(cmul3[:, 2:3], INV_QA)
            s3 = scal.tile([P, 3], F32)
            nc.vector.tensor_tensor(out=s3[:], in0=bc3[:], in1=cmul3[:], op=ALU.mult)
            nc.vector.tensor_scalar(s3[:], s3[:], EPS, None, op0=ALU.max)
            inv3 = scal.tile([P, 3], F32)
            nc.vector.reciprocal(inv3[:], s3[:])
            s_x, s_w, s_b = s3[:, 0:1], s3[:, 1:2], s3[:, 2:3]
            inv_sx, inv_sw, inv_sb = inv3[:, 0:1], inv3[:, 1:2], inv3[:, 2:3]
            s_xw = scal.tile([P, 1], F32)
            nc.vector.tensor_tensor(out=s_xw[:], in0=s_x, in1=s_w, op=ALU.mult)

            bsh = scal.tile([P, n_m], F32)
            nc.gpsimd.dma_start(bsh[:], bs_d.rearrange("(a p) -> p a", p=P))
            bqi = scal.tile([P, n_m], I16)
            nc.scalar.activation(bqi[:], bsh[:], AF.Identity, scale=inv_sb)
            bq = scal.tile([P, n_m], F32)
            nc.scalar.activation(bq[:], bqi[:], AF.Identity, scale=s_b)

            # ---------------- Phase 1: quantize x -> qxT (bf16) ------------
            # kt-major strips [128, 1024]; pass1 alternates ACT/DVE, pass2 DVE.
            for kt in range(n_kt):
                t = xsp.tile([P, TS], F32, tag="xf", name=f"xf{kt}")
                nc.sync.dma_start(t[:], xr[:, kt, :])
                mid = xmp.tile([P, TS], I16, tag="xm", name=f"xm{kt}")
                if kt % 2 == 0:
                    nc.scalar.activation(mid[:], t[:], AF.Identity, scale=inv_sx)
                else:
                    nc.vector.tensor_scalar(mid[:], t[:], inv_sx, None, op0=ALU.mult)
                nc.vector.tensor_copy(out=qxT[:, kt, :], in_=mid[:])

            # ---------------- Phase 2: stream w, quantize, matmul ----------
            def w_prep(m):
                qw = qwp.tile([P, n_kt, P], BF16, tag="qw", name=f"qw{m}")
                for h in range(2):
                    t = wrp.tile([P, HC, P], F32, tag="wr", name=f"wc{m}_{h}")
                    nc.sync.dma_start(t[:], whr[m, :, h])
                    mid = wmp.tile([P, HC, P], I16, tag="wm", name=f"wm{m}_{h}")
                    nc.scalar.activation(
                        mid[:].rearrange("k a b -> k (a b)"),
                        t[:].rearrange("k a b -> k (a b)"),
                        AF.Identity, scale=inv_sw,
                    )
                    nc.vector.tensor_copy(
                        out=qw[:, h * HC : (h + 1) * HC, :], in_=mid[:]
                    )
                return qw

            def evict(m, ps):
                for tb in range(n_tb):
                    nc.scalar.activation(
                        opre[:, m, tb * TB : (tb + 1) * TB], ps[tb][:],
                        AF.Identity, bias=bq[:, m : m + 1], scale=s_xw[:],
                    )

            # First 4 m-tiles run kt-outer across all 8 PSUM banks: PE demand
            # per x-strip stays below the strip DMA rate, so the PE does not
            # stall while qxT streams in.
            NB = 2
            qws = [w_prep(m) for m in range(NB)]
            psb = [
                [mmps.tile([P, TB], F32, tag="mm", name=f"psb{m}_{tb}") for tb in range(n_tb)]
                for m in range(NB)
            ]
            for kt in range(n_kt):
                for m in range(NB):
                    for tb in range(n_tb):
                        nc.tensor.matmul(
                            psb[m][tb][:],
                            lhsT=qws[m][:, kt, :],
                            rhs=qxT[:, kt, tb * TB : (tb + 1) * TB],
                            start=(kt == 0),
                            stop=(kt == n_kt - 1),
                        )
            for m in range(NB):
                evict(m, psb[m])

            for m in range(NB, n_m):
                qw = w_prep(m)
                ps = [
                    mmps.tile([P, TB], F32, tag="mm", name=f"ps{m}_{tb}")
                    for tb in range(n_tb)
                ]
                for kt in range(n_kt):
                    for tb in range(n_tb):
                        nc.tensor.matmul(
                            ps[tb][:],
                            lhsT=qw[:, kt, :],
                            rhs=qxT[:, kt, tb * TB : (tb + 1) * TB],
                            start=(kt == 0),
                            stop=(kt == n_kt - 1),
                        )
                evict(m, ps)

            omax = scal.tile([P, n_m], F32)
            for m in range(n_m):
                nc.vector.tensor_reduce(
                    omax[:, m : m + 1], opre[:, m, :], axis=AX.X, op=ALU.max,
                    apply_absolute_value=True,
                )

            # ---------------- Phase 3: out absmax -> requantize ------------
            om1 = scal.tile([P, 1], F32)
            nc.vector.tensor_reduce(om1[:], omax[:], axis=AX.X, op=ALU.max)
            omr = scal.tile([P, 1], F32)
            nc.gpsimd.partition_all_reduce(
                omr[:], om1[:], channels=P, reduce_op=bass_isa.ReduceOp.max
            )
            nc.sync.dma_start(cc2_in[:], omr[:1, :])
            nc.gpsimd.collective_compute(
                "AllReduce", ALU.max, replica_groups=groups,
                ins=[cc2_in[:]], outs=[cc2_out[:]],
            )
            go = scal.tile([P, 1], F32)
            nc.sync.dma_start(go[:1, :], cc2_out[:])
            bco = scal.tile([P, 1], F32)
            nc.gpsimd.partition_broadcast(bco[:], go[:1, :], channels=P)
            s_o = scal.tile([P, 1], F32)
            nc.vector.tensor_scalar(s_o[:], bco[:], INV_QA, EPS, op0=ALU.mult, op1=ALU.max)
            inv_so = scal.tile([P, 1], F32)
            nc.vector.reciprocal(inv_so[:], s_o[:])

            for m in range(n_m):
                oi = xmp.tile([P, TS], I16, tag="xm", name=f"oi{m}")
                nc.vector.tensor_scalar(oi[:], opre[:, m, :], inv_so[:], None, op0=ALU.mult)
                # requantized values overwrite opre in place, then stream out
                nc.scalar.activation(opre[:, m, :], oi[:], AF.Identity, scale=s_o[:])
                nc.sync.dma_start(o_d[m * P : (m + 1) * P, :], opre[:, m, :])

    nc.compile()
    return nc


def _run(nc, inputs, n_cores, T, K, J, trace=False):
    from concourse.bass_utils import run_bass_kernel_spmd

    TS, JS = T // N_TP, J // N_JP
    n_m = JS // P
    n_kt = K // P
    x = np.ascontiguousarray(inputs["x"], dtype=np.float32)
    w = np.ascontiguousarray(inputs["weight"], dtype=np.float32)
    b = np.ascontiguousarray(inputs["b"], dtype=np.float32)
    xT = np.ascontiguousarray(x.T)  # [K, T]
    in_maps = []
    for c in range(n_cores):
        ti, jb = c // N_JP, c % N_JP
        # roll so the core's exclusive absmax slice is always columns [0,512)
        xs = np.roll(xT[:, ti * TS : (ti + 1) * TS], -512 * jb, axis=1)
        wsh = np.roll(w[jb * JS : (jb + 1) * JS, :], -512 * ti, axis=0)
        # [m, k, kt, j]: chunk m contiguous, 16-KB runs per partition k
        wh = np.ascontiguousarray(
            wsh.reshape(n_m, P, n_kt, P).transpose(0, 3, 2, 1)
        )
        in_maps.append(
            {
                "xq": np.ascontiguousarray(xs),
                "wh": wh,
                "b_full": b,
                "b_shard": np.ascontiguousarray(
                    np.roll(b[jb * JS : (jb + 1) * JS], -512 * ti)
                ),
            }
        )
    res = run_bass_kernel_spmd(nc, in_maps, core_ids=list(range(n_cores)), trace=trace)
    out = np.empty((T, J), dtype=np.float32)
    for c in range(n_cores):
        ti, jb = c // N_JP, c % N_JP
        oT = res.results[c]["outT"]  # [JS, TS], rolled in both dims
        oT = np.roll(oT, 512 * ti, axis=0)
        oT = np.roll(oT, 512 * jb, axis=1)
        out[ti * TS : (ti + 1) * TS, jb * JS : (jb + 1) * JS] = oT.T
    return out, res


_NC_CACHE = {}


def kernel(**inputs) -> np.ndarray:
    n_cores, T, K, J = 8, 4096, 4096, 4096
    key = (n_cores, T, K, J)
    if key not in _NC_CACHE:
        _NC_CACHE[key] = build(n_cores, T, K, J)
    out, _ = _run(_NC_CACHE[key], inputs, n_cores, T, K, J)
    return out


# revision 24
# speedup vs baseline: 1.0246x; 1.0214x over previous
"""Quantized Linear (8-bit act / 4-bit weight fake-quant) on 8 Trainium2 cores.

Math (per reference):
  xq = rne(x / s_x) * s_x          s_x = max(absmax(x)/127, 1e-8)
  wq = rne(w / s_w) * s_w          s_w = max(absmax(w)/7,   1e-8)
  bq = rne(b / s_b) * s_b          s_b = max(absmax(b)/127, 1e-8)
  out_pre = bq + xq @ wq.T
  out = rne(out_pre / s_o) * s_o   s_o = max(absmax(out_pre)/127, 1e-8)

Device strategy (4x2 grid: tokens x out_features, 8 cores):
  - Host feeds each core k-major slabs (xT slices; w as [m, k, kt, j] blocks
    so each 2-MiB weight chunk moves with 16-KB descriptors). No PE
    transposes: quantized tiles land directly in [k_partition, free] layout
    (lhsT = w tile, rhs = xT strip, out = outT).
  - Quantization uses the hardware fp32->int16 convert (round-to-nearest-
    even, matches jnp.round): pass1 = x*inv_s -> int16, pass2 = int16 ->
    bf16 copy (exact for |q| <= 127). Integer bf16 matmul accumulates
    exactly in fp32 PSUM; scales fold into the PSUM eviction.
  - Per-core exclusive absmax slices (first 512 columns of each slab, made
    uniform across cores by host-side np.roll) + one tiny AllReduce-max for
    (x, w); a second one for out_pre before the final requantization.
  - Each core computes outT[j_shard, t_shard] = [2048, 1024]; host
    un-rolls and reassembles.
"""

import sys

sys.path.insert(0, "/opt/trn_rl_repo")

import numpy as np

import concourse.bass as bass
import concourse.mybir as mybir
import concourse.tile as tile
from concourse import bacc, bass_isa

F32 = mybir.dt.float32
BF16 = mybir.dt.bfloat16
I16 = mybir.dt.int16
AF = mybir.ActivationFunctionType
ALU = mybir.AluOpType
AX = mybir.AxisListType

EPS = 1e-8
INV_QA = float(np.float32(1.0) / np.float32(127.0))
INV_QW = float(np.float32(1.0) / np.float32(7.0))

P = 128
N_TP = 4  # token-parallel degree
N_JP = 2  # feature-parallel degree


def build(n_cores=8, T=4096, K=4096, J=4096, TB=512):
    assert n_cores == N_TP * N_JP
    TS, JS = T // N_TP, J // N_JP  # 1024 tokens, 2048 features per core
    n_kt = K // P  # 32 contraction tiles
    n_m = JS // P  # 16 output-feature tiles
    n_tb = TS // TB  # 2 psum column groups

    nc = bacc.Bacc(
        "TRN2", target_bir_lowering=False, debug=False, num_devices=n_cores
    )

    xq_d = nc.dram_tensor("xq", [K, TS], F32, kind="ExternalInput")
    wh_d = nc.dram_tensor("wh", [n_m, P, n_kt, P], F32, kind="ExternalInput")
    b_d = nc.dram_tensor("b_full", [J], F32, kind="ExternalInput")
    bs_d = nc.dram_tensor("b_shard", [JS], F32, kind="ExternalInput")
    o_d = nc.dram_tensor("outT", [JS, TS], F32, kind="ExternalOutput")
    cc1_in = nc.dram_tensor("cc1_in", [1, 3], F32)
    cc1_out = nc.dram_tensor("cc1_out", [1, 3], F32)
    cc2_in = nc.dram_tensor("cc2_in", [1, 1], F32)
    cc2_out = nc.dram_tensor("cc2_out", [1, 1], F32)
    groups = [list(range(n_cores))]

    xr = xq_d.rearrange("(kt k) t -> k kt t", k=P)

    with tile.TileContext(nc) as tc:
        with (
            tc.tile_pool(name="big", bufs=1) as big,
            tc.tile_pool(name="scal", bufs=1) as scal,
            tc.tile_pool(name="xslp", bufs=3) as xslp,
            tc.tile_pool(name="xsp", bufs=3) as xsp,
            tc.tile_pool(name="xmp", bufs=3) as xmp,
            tc.tile_pool(name="wrp", bufs=2) as wrp,
            tc.tile_pool(name="wmp", bufs=2) as wmp,
            tc.tile_pool(name="qwp", bufs=3) as qwp,
            tc.tile_pool(name="mmps", bufs=8, space="PSUM") as mmps,
        ):
            qxT = big.tile([P, n_kt, TS], BF16)  # 64 KB/partition
            opre = big.tile([P, n_m, TS], F32)  # 64 KB/partition

            # ---------------- Phase 0: absmax of exclusive slices ----------
            # Host rolls each slab so this core's exclusive slice is always
            # columns [0, 512) of xq and chunks [0, 4) of wh.
            HC = n_kt // 2  # w half-chunk: 16 kt rows
            whr = wh_d.rearrange("m k (h kt) j -> m k h kt j", h=2)
            n_xs = 16  # x-slice strips of [128, 2kt, 512]
            am = scal.tile([P, n_xs + 8 + 1], F32)
            xsl = xr[:, :, 0:512].rearrange("k (g t) c -> k g t c", g=n_xs)
            for g in range(n_xs):
                t = xslp.tile([P, 2, 512], F32, tag="xs", name=f"xs{g}")
                eng = nc.sync if g % 2 == 0 else nc.gpsimd
                eng.dma_start(t[:], xsl[:, g])
                if g < 8:
                    w = wrp.tile([P, HC, P], F32, tag="wr", name=f"ws{g}")
                    nc.scalar.dma_start(w[:], whr[g // 2, :, g % 2])
                    # interleave x/w reduces: ready w data fills x-DMA waits
                    nc.vector.tensor_reduce(
                        am[:, n_xs + g : n_xs + g + 1],
                        w[:].rearrange("k a b -> k (a b)"),
                        axis=AX.X, op=ALU.max, apply_absolute_value=True,
                    )
                nc.vector.tensor_reduce(
                    am[:, g : g + 1], t[:].rearrange("k a b -> k (a b)"),
                    axis=AX.X, op=ALU.max, apply_absolute_value=True,
                )
            bfull = scal.tile([P, J // P], F32)
            nc.gpsimd.dma_start(bfull[:], b_d.rearrange("(p a) -> p a", p=P))
            nc.vector.tensor_reduce(
                am[:, n_xs + 8 :], bfull[:], axis=AX.X, op=ALU.max,
                apply_absolute_value=True,
            )

            m3 = scal.tile([P, 3], F32)
            nc.vector.tensor_reduce(
                m3[:, 0:1], am[:, 0:n_xs], axis=AX.X, op=ALU.max
            )
            nc.vector.tensor_reduce(
                m3[:, 1:2], am[:, n_xs : n_xs + 8], axis=AX.X, op=ALU.max
            )
            nc.vector.tensor_copy(out=m3[:, 2:3], in_=am[:, n_xs + 8 :])
            g3 = scal.tile([P, 3], F32)
            nc.gpsimd.partition_all_reduce(
                g3[:], m3[:], channels=P, reduce_op=bass_isa.ReduceOp.max
            )
            nc.sync.dma_start(cc1_in[:], g3[:1, :])
            nc.gpsimd.collective_compute(
                "AllReduce", ALU.max, replica_groups=groups,
                ins=[cc1_in[:]], outs=[cc1_out[:]],
            )
            gx = scal.tile([P, 3], F32)
            nc.sync.dma_start(gx[:1, :], cc1_out[:])
            bc3 = scal.tile([P, 3], F32)
            nc.gpsimd.partition_broadcast(bc3[:], gx[:1, :], channels=P)

            # fused scale chain, single engine (DVE), minimal cross-engine hops
            cmul3 = scal.tile([P, 3], F32)
            nc.vector.memset(cmul3[:, 0:1], INV_QA)
            nc.vector.memset(cmul3[:, 1:2], INV_QW)
            nc.vector.memset(cmul3[:, 2:3], INV_QA)
            s3 = scal.tile([P, 3], F32)
            nc.vector.tensor_tensor(out=s3[:], in0=bc3[:], in1=cmul3[:], op=ALU.mult)
            nc.vector.tensor_scalar(s3[:], s3[:], EPS, None, op0=ALU.max)
            inv3 = scal.tile([P, 3], F32)
            nc.vector.reciprocal(inv3[:], s3[:])
            s_x, s_w, s_b = s3[:, 0:1], s3[:, 1:2], s3[:, 2:3]
            inv_sx, inv_sw, inv_sb = inv3[:, 0:1], inv3[:, 1:2], inv3[:, 2:3]
            s_xw = scal.tile([P, 1], F32)
            nc.vector.tensor_tensor(out=s_xw[:], in0=s_x, in1=s_w, op=ALU.mult)

            bsh = scal.tile([P, n_m], F32)
            nc.gpsimd.dma_start(bsh[:], bs_d.rearrange("(a p) -> p a", p=P))
            bqi = scal.tile([P, n_m], I16)
            nc.scalar.activation(bqi[:], bsh[:], AF.Identity, scale=inv_sb)
            bq = scal.tile([P, n_m], F32)
            nc.scalar.activation(bq[:], bqi[:], AF.Identity, scale=s_b)

            # ---------------- Phase 1: quantize x -> qxT (bf16) ------------
            # kt-major strips [128, 1024]; pass1 alternates ACT/DVE, pass2 DVE.
            for kt in range(n_kt):
                t = xsp.tile([P, TS], F32, tag="xf", name=f"xf{kt}")
                nc.sync.dma_start(t[:], xr[:, kt, :])
                mid = xmp.tile([P, TS], I16, tag="xm", name=f"xm{kt}")
                if kt % 2 == 0:
                    nc.scalar.activation(mid[:], t[:], AF.Identity, scale=inv_sx)
                else:
                    nc.vector.tensor_scalar(mid[:], t[:], inv_sx, None, op0=ALU.mult)
                nc.vector.tensor_copy(out=qxT[:, kt, :], in_=mid[:])

            # ---------------- Phase 2: stream w, quantize, matmul ----------
            def w_prep(m):
                qw = qwp.tile([P, n_kt, P], BF16, tag="qw", name=f"qw{m}")
                for h in range(2):
                    t = wrp.tile([P, HC, P], F32, tag="wr", name=f"wc{m}_{h}")
                    nc.sync.dma_start(t[:], whr[m, :, h])
                    mid = wmp.tile([P, HC, P], I16, tag="wm", name=f"wm{m}_{h}")
                    nc.scalar.activation(
                        mid[:].rearrange("k a b -> k (a b)"),
                        t[:].rearrange("k a b -> k (a b)"),
                        AF.Identity, scale=inv_sw,
                    )
                    nc.vector.tensor_copy(
                        out=qw[:, h * HC : (h + 1) * HC, :], in_=mid[:]
                    )
                return qw

            def evict(m, ps):
                for tb in range(n_tb):
                    nc.scalar.activation(
                        opre[:, m, tb * TB : (tb + 1) * TB], ps[tb][:],
                        AF.Identity, bias=bq[:, m : m + 1], scale=s_xw[:],
                    )

            # First 4 m-tiles run kt-outer across all 8 PSUM banks: PE demand
            # per x-strip stays below the strip DMA rate, so the PE does not
            # stall while qxT streams in.
            NB = 2
            qws = [w_prep(m) for m in range(NB)]
            psb = [
                [mmps.tile([P, TB], F32, tag="mm", name=f"psb{m}_{tb}") for tb in range(n_tb)]
                for m in range(NB)
            ]
            for kt in range(n_kt):
                for m in range(NB):
                    for tb in range(n_tb):
                        nc.tensor.matmul(
                            psb[m][tb][:],
                            lhsT=qws[m][:, kt, :],
                            rhs=qxT[:, kt, tb * TB : (tb + 1) * TB],
                            start=(kt == 0),
                            stop=(kt == n_kt - 1),
                        )
            for m in range(NB):
                evict(m, psb[m])

            for m in range(NB, n_m):
                qw = w_prep(m)
                ps = [
                    mmps.tile([P, TB], F32, tag="mm", name=f"ps{m}_{tb}")
                    for tb in range(n_tb)
                ]
                for kt in range(n_kt):
                    for tb in range(n_tb):
                        nc.tensor.matmul(
                            ps[tb][:],
                            lhsT=qw[:, kt, :],
                            rhs=qxT[:, kt, tb * TB : (tb + 1) * TB],
                            start=(kt == 0),
                            stop=(kt == n_kt - 1),
                        )
                evict(m, ps)

            omax = scal.tile([P, n_m], F32)
            for m in range(n_m):
                nc.vector.tensor_reduce(
                    omax[:, m : m + 1], opre[:, m, :], axis=AX.X, op=ALU.max,
                    apply_absolute_value=True,
                )

            # ---------------- Phase 3: out absmax -> requantize ------------
            om1 = scal.tile([P, 1], F32)
            nc.vector.tensor_reduce(om1[:], omax[:], axis=AX.X, op=ALU.max)
            omr = scal.tile([P, 1], F32)
            nc.gpsimd.partition_all_reduce(
                omr[:], om1[:], channels=P, reduce_op=bass_isa.ReduceOp.max
            )
            nc.sync.dma_start(cc2_in[:], omr[:1, :])
            nc.gpsimd.collective_compute(
                "AllReduce", ALU.max, replica_groups=groups,
                ins=[cc2_in[:]], outs=[cc2_out[:]],
            )
            go = scal.tile([P, 1], F32)
            nc.sync.dma_start(go[:1, :], cc2_out[:])
            bco = scal.tile([P, 1], F32)
            nc.gpsimd.partition_broadcast(bco[:], go[:1, :], channels=P)
            s_o = scal.tile([P, 1], F32)
            nc.vector.tensor_scalar(s_o[:], bco[:], INV_QA, EPS, op0=ALU.mult, op1=ALU.max)
            inv_so = scal.tile([P, 1], F32)
            nc.vector.reciprocal(inv_so[:], s_o[:])

            for m in range(n_m):
                oi = xmp.tile([P, TS], I16, tag="xm", name=f"oi{m}")
                nc.vector.tensor_scalar(oi[:], opre[:, m, :], inv_so[:], None, op0=ALU.mult)
                # requantized values overwrite opre in place, then stream out
                nc.scalar.activation(opre[:, m, :], oi[:], AF.Identity, scale=s_o[:])
                nc.sync.dma_start(o_d[m * P : (m + 1) * P, :], opre[:, m, :])

    nc.compile()
    return nc


def _run(nc, inputs, n_cores, T, K, J, trace=False):
    from concourse.bass_utils import run_bass_kernel_spmd

    TS, JS = T // N_TP, J // N_JP
    n_m = JS // P
    n_kt = K // P
    x = np.ascontiguousarray(inputs["x"], dtype=np.float32)
    w = np.ascontiguousarray(inputs["weight"], dtype=np.float32)
    b = np.ascontiguousarray(inputs["b"], dtype=np.float32)
    xT = np.ascontiguousarray(x.T)  # [K, T]
    in_maps = []
    for c in range(n_cores):
        ti, jb = c // N_JP, c % N_JP
        # roll so the core's exclusive absmax slice is always columns [0,512)
        xs = np.roll(xT[:, ti * TS : (ti + 1) * TS], -512 * jb, axis=1)
        wsh = np.roll(w[jb * JS : (jb + 1) * JS, :], -512 * ti, axis=0)
        # [m, k, kt, j]: chunk m contiguous, 16-KB runs per partition k
        wh = np.ascontiguousarray(
            wsh.reshape(n_m, P, n_kt, P).transpose(0, 3, 2, 1)
        )
        in_maps.append(
            {
                "xq": np.ascontiguousarray(xs),
                "wh": wh,
                "b_full": b,
                "b_shard": np.ascontiguousarray(
                    np.roll(b[jb * JS : (jb + 1) * JS], -512 * ti)
                ),
            }
        )
    res = run_bass_kernel_spmd(nc, in_maps, core_ids=list(range(n_cores)), trace=trace)
    out = np.empty((T, J), dtype=np.float32)
    for c in range(n_cores):
        ti, jb = c // N_JP, c % N_JP
        oT = res.results[c]["outT"]  # [JS, TS], rolled in both dims
        oT = np.roll(oT, 512 * ti, axis=0)
        oT = np.roll(oT, 512 * jb, axis=1)
        out[ti * TS : (ti + 1) * TS, jb * JS : (jb + 1) * JS] = oT.T
    return out, res


_NC_CACHE = {}


def kernel(**inputs) -> np.ndarray:
    n_cores, T, K, J = 8, 4096, 4096, 4096
    key = (n_cores, T, K, J)
    if key not in _NC_CACHE:
        _NC_CACHE[key] = build(n_cores, T, K, J)
    out, _ = _run(_NC_CACHE[key], inputs, n_cores, T, K, J)
    return out
